# revision 9
# baseline (speedup 1.0000x reference)
"""Distributed Trainium2 Bass kernel for multi-head attention w/ RoPE.

Reference op (B=4, S=2048, D=1024, H=16, HD=64, fp32):
    q/k/v = hidden @ W{q,k,v}.T + b   (per-head reshape)
    q, k  = rope(q), rope(k)
    out   = softmax(q k^T / sqrt(HD)) v  @ Wo.T

Sharding: 8 cores = 4 batches x 2 query-halves. Each core computes the
K/V projections for its whole batch (duplicated across the half-pair --
this avoids every collective), Q projection + attention for its own 1024
queries, and the o-projection for its own output rows. Host-side unshard
is a pure concat.

Device layout is fully transposed (features on partitions): projections
produce Q^T/K^T, scores are computed as S^T = (K^T_h)^T-chunks x Q^T_h,
exp runs on ACT with the 1/sqrt(HD) scale folded in, attn@V uses natural
V (from an x^T-stationary projection) augmented with a ones column so the
softmax denominator falls out of the same matmul, and normalization is
folded into the attn-out eviction via a DMA partition-broadcast of the
reciprocal row. RoPE = 2 DVE muls (cos table, sign-folded sin table) + 4
band adds straight out of PSUM. Biases enter through an augmented K=1
contraction row (ones x bias), so any bias values are handled.
"""

import sys

import numpy as np

try:  # concourse ships in the container; fall back to the staged repo
    import concourse.bass  # noqa: F401
except Exception:  # pragma: no cover
    sys.path.insert(0, "/opt/trn_rl_repo")

import ml_dtypes

B, S, D, H = 4, 2048, 1024, 16
HD = D // H                      # 64
P = 128
NCORES = 8
SQ = S // 2                      # 1024 queries per core
SK = S                           # 2048 keys per core
ND = D // P                      # 8 feature chunks
NT = SK // P                     # 16 key/token chunks
QF = 512                         # matmul moving width
NQF = SQ // QF                   # 2
ROPE_BASE = 10000.0
BF16 = ml_dtypes.bfloat16

TRACE = False                    # test harness flips this
TRACE_KW = {}
LAST = {}                        # exec_time_ns / trace path for test harness

_cache = {}


def _build_nc():
    import concourse.bass as bass
    import concourse.mybir as mybir
    import concourse.tile as tile
    from concourse import bacc
    from contextlib import ExitStack

    f32 = mybir.dt.float32
    bf16 = mybir.dt.bfloat16
    AF = mybir.ActivationFunctionType
    PSUM = bass.MemorySpace.PSUM

    nc = bacc.Bacc(None)
    xT = nc.declare_dram_parameter("xT", [D + 1, SK], bf16, False)
    wqT = nc.declare_dram_parameter("wqT", [D + 1, D], bf16, False)
    wkT = nc.declare_dram_parameter("wkT", [D + 1, D], bf16, False)
    wvT = nc.declare_dram_parameter("wvT", [D + 1, D], bf16, False)
    woT = nc.declare_dram_parameter("woT", [D, D], bf16, False)
    cosq = nc.declare_dram_parameter("cosq", [P, SQ], f32, False)
    sinq = nc.declare_dram_parameter("sinq", [P, SQ], f32, False)
    cosk = nc.declare_dram_parameter("cosk", [P, SK], f32, False)
    sink = nc.declare_dram_parameter("sink", [P, SK], f32, False)
    out = nc.declare_dram_parameter("out", [SQ, D], f32, True)

    with tile.TileContext(nc) as tc, ExitStack() as st:
        persist = st.enter_context(tc.tile_pool(name="persist", bufs=1))
        qt = [persist.tile([P, SQ], bf16, tag=f"qt{i}", name=f"qt{i}") for i in range(ND)]
        kt = [persist.tile([P, SK], bf16, tag=f"kt{i}", name=f"kt{i}") for i in range(ND)]
        vst = [persist.tile([P, H, HD + 1], bf16, tag=f"v{t}", name=f"v{t}") for t in range(NT)]
        at = [persist.tile([P, SQ], bf16, tag=f"at{i}", name=f"at{i}") for i in range(ND)]

        # ---------------- phase 1: projections + RoPE -------------------
        with ExitStack() as p1:
            sb1 = p1.enter_context(tc.tile_pool(name="ph1", bufs=1))
            wpool = p1.enter_context(tc.tile_pool(name="wp", bufs=1))
            tpool = p1.enter_context(tc.tile_pool(name="tmp", bufs=3))
            ps1 = p1.enter_context(tc.tile_pool(name="ps1", bufs=4, space=PSUM))

            xs = [sb1.tile([P, SK], bf16, tag=f"x{d}", name=f"x{d}") for d in range(ND)]
            for d_ in range(ND):
                nc.sync.dma_start(out=xs[d_][:], in_=xT[d_ * P:(d_ + 1) * P, :])
            xone = sb1.tile([1, SK], bf16, tag="xone", name="xone")
            nc.sync.dma_start(out=xone[:], in_=xT[D:D + 1, :])

            cq = sb1.tile([P, SQ], f32, tag="cq", name="cq")
            sq_ = sb1.tile([P, SQ], f32, tag="sq", name="sq")
            ck = sb1.tile([P, SK], f32, tag="ck", name="ck")
            sk_ = sb1.tile([P, SK], f32, tag="sk", name="sk")
            nc.sync.dma_start(out=cq[:], in_=cosq[:, :])
            nc.sync.dma_start(out=sq_[:], in_=sinq[:, :])
            nc.sync.dma_start(out=ck[:], in_=cosk[:, :])
            nc.sync.dma_start(out=sk_[:], in_=sink[:, :])

            def qk_proj(wdram, outtiles, tabc, tabs, ntok):
                """outtiles[p][o, t] = rope(W @ x^T + b) for o-chunk p."""
                w = wpool.tile([P, ND, D], bf16, tag="w", name="w")
                wb = wpool.tile([1, D], bf16, tag="wb", name="wb")
                nc.sync.dma_start(
                    out=w[:], in_=wdram[0:D, :].rearrange("(n p) o -> p n o", p=P))
                nc.sync.dma_start(out=wb[:], in_=wdram[D:D + 1, :])
                for p_ in range(ND):
                    for c in range(ntok // QF):
                        ps = ps1.tile([P, QF], f32, tag="pp", name="pp")
                        for d_ in range(ND):
                            nc.tensor.matmul(
                                ps[:], w[:, d_, p_ * P:(p_ + 1) * P],
                                xs[d_][:, c * QF:(c + 1) * QF],
                                start=(d_ == 0), stop=False)
                        nc.tensor.matmul(
                            ps[:], wb[:, p_ * P:(p_ + 1) * P],
                            xone[:, c * QF:(c + 1) * QF],
                            start=False, stop=True)
                        # t2 lives in PSUM: the partition-crossed add below
                        # is only legal with the shifted operand on the
                        # PSUM port (SBUF operands must share out's base).
                        t1 = tpool.tile([P, QF], f32, tag="t1", name="t1")
                        t2 = ps1.tile([P, QF], f32, tag="t2p", name="t2p", bufs=3)
                        cs = tabc[:, c * QF:(c + 1) * QF]
                        sn = tabs[:, c * QF:(c + 1) * QF]
                        nc.vector.tensor_mul(t1[:], ps[:], cs)
                        nc.vector.tensor_mul(t2[:], ps[:], sn)
                        o = outtiles[p_]
                        cslice = slice(c * QF, (c + 1) * QF)
                        for b0 in (0, 64):
                            nc.vector.tensor_add(
                                o[b0:b0 + 32, cslice],
                                t1[b0:b0 + 32, :], t2[b0 + 32:b0 + 64, :])
                            nc.vector.tensor_add(
                                o[b0 + 32:b0 + 64, cslice],
                                t1[b0 + 32:b0 + 64, :], t2[b0:b0 + 32, :])

            qk_proj(wqT, qt, cq, sq_, SQ)
            qk_proj(wkT, kt, ck, sk_, SK)

            # V in natural layout [tokens, feat] (x^T chunks stationary)
            wv = wpool.tile([P, ND, D], bf16, tag="w", name="w")
            wvb = wpool.tile([1, D], bf16, tag="wb", name="wb")
            nc.sync.dma_start(
                out=wv[:], in_=wvT[0:D, :].rearrange("(n p) o -> p n o", p=P))
            nc.sync.dma_start(out=wvb[:], in_=wvT[D:D + 1, :])
            for t_ in range(NT):
                for oh in range(2):
                    ps = ps1.tile([P, QF], f32, tag="pp", name="pp")
                    for d_ in range(ND):
                        nc.tensor.matmul(
                            ps[:], xs[d_][:, t_ * P:(t_ + 1) * P],
                            wv[:, d_, oh * QF:(oh + 1) * QF],
                            start=(d_ == 0), stop=False)
                    nc.tensor.matmul(
                        ps[:], xone[:, t_ * P:(t_ + 1) * P],
                        wvb[:, oh * QF:(oh + 1) * QF],
                        start=False, stop=True)
                    nc.scalar.activation(
                        vst[t_][:, oh * 8:(oh + 1) * 8, 0:HD],
                        ps[:].rearrange("p (h d) -> p h d", d=HD), AF.Copy)
                nc.vector.memset(vst[t_][:, :, HD:HD + 1], 1.0)

        # ---------------- phase 2: attention ---------------------------
        with ExitStack() as p2:
            etp = p2.enter_context(tc.tile_pool(name="et", bufs=40))
            npool = p2.enter_context(tc.tile_pool(name="nrm", bufs=4))
            ps_s = p2.enter_context(tc.tile_pool(name="pss", bufs=4, space=PSUM))
            ps_o = p2.enter_context(tc.tile_pool(name="pso", bufs=3, space=PSUM))
            for h in range(H):
                pi, b0 = h // 2, 64 * (h % 2)
                for qh in range(NQF):
                    qs = slice(qh * QF, (qh + 1) * QF)
                    ets = []
                    for kc in range(NT):
                        sp = ps_s.tile([P, QF], f32, tag="s", name="s")
                        nc.tensor.matmul(
                            sp[:], kt[pi][b0:b0 + 64, kc * P:(kc + 1) * P],
                            qt[pi][b0:b0 + 64, qs], start=True, stop=True)
                        e = etp.tile([P, QF], bf16, tag="e", name="e")
                        nc.scalar.activation(e[:], sp[:], AF.Exp, scale=0.125)
                        ets.append(e)
                    op = ps_o.tile([HD + 1, QF], f32, tag="o", name="o")
                    for kc in range(NT):
                        nc.tensor.matmul(
                            op[:], vst[kc][:, h, :], ets[kc][:],
                            start=(kc == 0), stop=(kc == NT - 1))
                    # reciprocal base-aligned at p64, DMA-hop to p0,
                    # broadcast, then normalize during the PSUM eviction
                    sm = npool.tile([HD + 1, QF], f32, tag="sm", name="sm")
                    nc.vector.reciprocal(sm[HD:HD + 1, :], op[HD:HD + 1, :])
                    rc = npool.tile([1, QF], f32, tag="rc", name="rc")
                    nc.sync.dma_start(out=rc[:], in_=sm[HD:HD + 1, :])
                    bc = npool.tile([HD, QF], f32, tag="bc", name="bc")
                    nc.gpsimd.partition_broadcast(bc[:], rc[:])
                    nc.vector.tensor_mul(
                        at[pi][b0:b0 + 64, qs], op[0:HD, :], bc[:])

        # ---------------- phase 3: o-projection ------------------------
        with ExitStack() as p3:
            wop = p3.enter_context(tc.tile_pool(name="wo", bufs=1))
            outp = p3.enter_context(tc.tile_pool(name="ou", bufs=4))
            ps3 = p3.enter_context(tc.tile_pool(name="ps3", bufs=4, space=PSUM))
            wo = wop.tile([P, ND, D], bf16, tag="wo", name="wo")
            nc.sync.dma_start(
                out=wo[:], in_=woT[:, :].rearrange("(n p) o -> p n o", p=P))
            for qc in range(ND):
                for oh in range(2):
                    ps = ps3.tile([P, QF], f32, tag="p3", name="p3")
                    for f in range(ND):
                        nc.tensor.matmul(
                            ps[:], at[f][:, qc * P:(qc + 1) * P],
                            wo[:, f, oh * QF:(oh + 1) * QF],
                            start=(f == 0), stop=(f == ND - 1))
                    ob = outp.tile([P, QF], f32, tag="ob", name="ob")
                    nc.scalar.activation(ob[:], ps[:], AF.Copy)
                    nc.sync.dma_start(
                        out=out[qc * P:(qc + 1) * P, oh * QF:(oh + 1) * QF],
                        in_=ob[:])
    nc.compile()
    return nc


def _rope_tables(pos):
    """pos [n] -> (cos [128, n] f32, sign-folded sin [128, n] f32)."""
    inv = ROPE_BASE ** (-np.arange(0, HD, 2, dtype=np.float64) / HD)
    fr = np.outer(pos.astype(np.float64), inv)          # [n, 32]
    c, s = np.cos(fr), np.sin(fr)
    cos64 = np.concatenate([c, c], axis=1).T            # [64, n]
    sinA = np.concatenate([s, -s], axis=1).T            # [64, n]
    return (np.tile(cos64, (2, 1)).astype(np.float32),
            np.tile(sinA, (2, 1)).astype(np.float32))


def _aug_w(w, b):
    """[D, D] weight + [D] bias -> bf16 [D+1, D] (W.T with bias row)."""
    wa = np.empty((D + 1, D), dtype=np.float32)
    wa[:D] = np.asarray(w, dtype=np.float32).T
    wa[D] = np.asarray(b, dtype=np.float32)
    return np.ascontiguousarray(wa).astype(BF16)


def kernel(hidden_states, position_ids, Wq, bq, Wk, bk, Wv, bv, Wo):
    from concourse import bass_utils

    if "nc" not in _cache:
        _cache["nc"] = _build_nc()
    nc = _cache["nc"]

    hs = np.asarray(hidden_states, dtype=np.float32)
    pos = np.asarray(position_ids)
    wq = _aug_w(Wq, bq)
    wk = _aug_w(Wk, bk)
    wv = _aug_w(Wv, bv)
    wo = np.ascontiguousarray(np.asarray(Wo, dtype=np.float32).T).astype(BF16)

    in_maps = []
    for core in range(NCORES):
        b, hf = core // 2, core % 2
        perm = np.concatenate([
            np.arange(hf * SQ, (hf + 1) * SQ),
            np.arange((1 - hf) * SQ, (2 - hf) * SQ)])
        xp = hs[b][perm]                                 # [S, D], own half first
        xT = np.empty((D + 1, SK), dtype=np.float32)
        xT[:D] = xp.T
        xT[D] = 1.0
        cq, sq = _rope_tables(np.asarray(pos[b][hf * SQ:(hf + 1) * SQ]))
        ck, sk = _rope_tables(np.asarray(pos[b][perm]))
        in_maps.append({
            "xT": xT.astype(BF16), "wqT": wq, "wkT": wk, "wvT": wv, "woT": wo,
            "cosq": cq, "sinq": sq, "cosk": ck, "sink": sk,
        })

    res = bass_utils.run_bass_kernel_spmd(
        nc, in_maps, core_ids=list(range(NCORES)), trace=TRACE, **TRACE_KW)
    LAST["exec_time_ns"] = res.exec_time_ns
    LAST["mean_exec_time_ns"] = res.mean_exec_time_ns
    LAST["trace"] = res.instructions_and_trace
    LAST["profile_json"] = res.profile_json

    outp = np.empty((B, S, D), dtype=np.float32)
    for core in range(NCORES):
        b, hf = core // 2, core % 2
        outp[b, hf * SQ:(hf + 1) * SQ] = res.results[core]["out"]
    return outp
